# revision 13
# baseline (speedup 1.0000x reference)
"""2D Haar DWT (level 1) Trainium2 Bass kernel — fp16 I/O.

Input  x: [16, 64, 256, 256] f32
Output y: [16, 256, 128, 128] f32, y[n, s*64+c, i, j] = Haar mix s of the
2x2 block x[n, c, 2i:2i+2, 2j:2j+2].

Sharding: pure data parallel over the batch dim — core k gets batches
[2k, 2k+2).

The transform is pure data movement (out bytes == in bytes), so the kernel
is HBM-bound. The rel-err budget (2e-2) admits fp16, halving traffic to
33.5 MB/core. The host:
  - scales x by 0.5 (exact power of two — folds the whole Haar
    normalization, so the device does pure +/- butterflies),
  - casts to fp16,
  - de-interleaves even/odd columns to [n, c, h, 2, 128] so the device
    butterflies are unit-stride (DVE 2x_1P mode needs 16-bit dtype +
    step 1 + 4B alignment),
  - un-scrambles the device's group-major output layout while upcasting.

Measured facts driving this version (v1-v6 traces):
  - DVE total is ~71 us (16.8M butterfly outputs, fp16 2x = 241 G elem/s)
    and is the on-chip critical path once DMA overlaps properly.
  - A single HWDGE ring streams at only ~300 GB/s; two rings together
    reach ~420 GB/s. So EVERY load and store is split in half across the
    sync and scalar rings, and each ring's FIFO order is
    [l0, l1, l2, s0, l3, s1, s2, s3] — all of g3's load bytes are queued
    before 3/4 of the stores, so the last load lands ~56 us, just ahead
    of when DVE needs it (v6 lost 8.6 us stalled on a late last load).
  - Stage 1 runs IN-PLACE in the input tile (s = a+b into the even rows,
    then d = s-2b via scalar_tensor_tensor into the odd rows), which
    removes the sd pool entirely -> bufs=3 on both pools (in 3x32 KB +
    out 3x32 KB = 192 KB/partition), so outpool WAR never gates stage 2.
  - First/last groups load in 1 MB chunks with per-chunk stage-1 ops so
    the head starts ~3 us after queue warmup and the tail is quartered.
  - GpSimd offload and mixed tile sizes both measured NEGATIVE (SBUF
    contention / pool-layout effects slow all DVE ops ~20-25%).

Layout per group of G=32 channels (Q = 128/G = 4 partitions per channel):
  it[p=(c,q), (o t j)] = x[n, c0 + p//Q, 64*(p%Q) + o, t, j]
  after stage 1: even rows u=0 hold s, odd rows u=1 hold d
  oadd tile = [subband 0 | subband 1], osub = [subband 2 | subband 3]
  output row i = q*G + r for row-pair r in [0, G).
"""

import sys

sys.path.insert(0, "/opt/trn_rl_repo")

import numpy as np

import concourse.bacc as bacc
import concourse.mybir as mybir
from concourse.tile import TileContext

N_CORES = 8
N_PER_CORE = 2  # batches per core
C = 64  # input channels
H = 256
W = 256
W2 = W // 2
G = 32  # channels per group
NG = C // G  # groups per batch item (2)
Q = 128 // G  # partitions per channel
F16 = mybir.dt.float16
FD = G * 512  # it-tile free elems (16384)
HD = FD // 2  # ring-half of a load (8192 elems = 2 MB)


def build_nc():
    nc = bacc.Bacc("TRN2", target_bir_lowering=False, debug=False)
    x = nc.dram_tensor("x", [N_PER_CORE, C, H, 2, W2], F16, kind="ExternalInput")
    y = nc.dram_tensor(
        "y", [N_PER_CORE, NG, 2, 128, G * 256], F16, kind="ExternalOutput"
    )

    groups = [(n, g) for n in range(N_PER_CORE) for g in range(NG)]
    n_grp = len(groups)

    with TileContext(nc) as tc:
        with (
            tc.tile_pool(name="inpool", bufs=3) as inpool,
            tc.tile_pool(name="outpool", bufs=3) as outpool,
        ):
            its = {}

            def emit_load(gi, chunks):
                """Load group gi, split across both rings; `chunks` 1 MB
                pieces per ring (1 = one 2 MB half per ring)."""
                n, g = groups[gi]
                it = inpool.tile([128, FD], F16, tag="in")
                its[gi] = it
                src = x[n, g * G : (g + 1) * G].rearrange(
                    "c (q o) t j -> (c q) (o t j)", q=Q
                )
                csz = HD // chunks
                for m in range(chunks):
                    for eng, base in ((nc.sync, 0), (nc.scalar, HD)):
                        lo = base + m * csz
                        eng.dma_start(
                            out=it[:, lo : lo + csz], in_=src[:, lo : lo + csz]
                        )

            def emit_stage1(gi, rs):
                """In-place vertical butterfly on row-pair range rs."""
                it = its[gi]
                itv = it[:].rearrange("p (r u f) -> p r u f", r=G, u=2)
                even, odd = itv[:, rs, 0], itv[:, rs, 1]
                nc.vector.tensor_add(out=even, in0=even, in1=odd)
                nc.vector.scalar_tensor_tensor(
                    out=odd,
                    in0=odd,
                    scalar=-2.0,
                    in1=even,
                    op0=mybir.AluOpType.mult,
                    op1=mybir.AluOpType.add,
                )

            def emit_stage2_and_stores(gi, quarters=False):
                n, g = groups[gi]
                it = its[gi]
                sdt = it[:].rearrange("p (r u t j) -> p u r t j", r=G, u=2, t=2)
                oadd = outpool.tile([128, G * 256], F16, tag="oadd")
                osub = outpool.tile([128, G * 256], F16, tag="osub")
                oav = oadd[:].rearrange("p (u r j) -> p u r j", u=2, r=G)
                osv = osub[:].rearrange("p (u r j) -> p u r j", u=2, r=G)
                half = G * 128  # elems per u-half of an output tile
                if not quarters:
                    nc.vector.tensor_add(
                        out=oav, in0=sdt[..., 0, :], in1=sdt[..., 1, :]
                    )
                    nc.vector.tensor_sub(
                        out=osv, in0=sdt[..., 0, :], in1=sdt[..., 1, :]
                    )
                    for e, t_ in enumerate((oadd, osub)):
                        nc.sync.dma_start(out=y[n, g, e, :, :half], in_=t_[:, :half])
                        nc.scalar.dma_start(out=y[n, g, e, :, half:], in_=t_[:, half:])
                else:
                    # tail: quarter ops + 0.5 MB stores, alternating rings
                    qf = half // 2
                    for k in range(4):
                        u, rh = k // 2, k % 2
                        rq = slice(rh * (G // 2), (rh + 1) * (G // 2))
                        nc.vector.tensor_add(
                            out=oav[:, u, rq],
                            in0=sdt[:, u, rq, 0],
                            in1=sdt[:, u, rq, 1],
                        )
                        nc.vector.tensor_sub(
                            out=osv[:, u, rq],
                            in0=sdt[:, u, rq, 0],
                            in1=sdt[:, u, rq, 1],
                        )
                        for (e, t_), eng in zip(
                            enumerate((oadd, osub)), (nc.sync, nc.scalar)
                        ):
                            eng.dma_start(
                                out=y[n, g, e, :, k * qf : (k + 1) * qf],
                                in_=t_[:, k * qf : (k + 1) * qf],
                            )

            # ---- explicit schedule: per-ring DMA order is
            # [l0, l1, l2, s0, l3, s1, s2, s3]; DVE order is g0..g3.
            emit_load(0, chunks=2)  # head: 4x 1 MB chunks (2 per ring)
            emit_load(1, chunks=1)
            emit_load(2, chunks=1)
            # g0 compute: per-chunk stage 1 (r-quarters matching the 1 MB
            # chunks: ring A carries r [0,16), ring B r [16,32))
            for m in range(2):
                emit_stage1(0, slice(m * 8, m * 8 + 8))
                emit_stage1(0, slice(16 + m * 8, 16 + m * 8 + 8))
            emit_stage2_and_stores(0)
            emit_load(3, chunks=2)
            emit_stage1(1, slice(0, G))
            emit_stage2_and_stores(1)
            emit_stage1(2, slice(0, G))
            emit_stage2_and_stores(2)
            # g3 compute: per-chunk stage 1, quartered stage 2 + stores
            for m in range(2):
                emit_stage1(3, slice(m * 8, m * 8 + 8))
                emit_stage1(3, slice(16 + m * 8, 16 + m * 8 + 8))
            emit_stage2_and_stores(3, quarters=True)

    nc.finalize()
    return nc


_NC = None


def _get_nc():
    global _NC
    if _NC is None:
        _NC = build_nc()
    return _NC


def _make_in_maps(x: np.ndarray) -> list[dict]:
    """Host prep: *0.5, cast fp16, de-interleave even/odd columns."""
    x = np.asarray(x)
    assert x.shape == (16, C, H, W), x.shape
    xr = x.reshape(16, C, H, W2, 2)
    xp = np.empty((16, C, H, 2, W2), dtype=np.float16)
    np.multiply(xr[..., 0], np.float32(0.5), out=xp[:, :, :, 0, :])
    np.multiply(xr[..., 1], np.float32(0.5), out=xp[:, :, :, 1, :])
    return [
        {"x": xp[k * N_PER_CORE : (k + 1) * N_PER_CORE]} for k in range(N_CORES)
    ]


def _gather(results: list[dict]) -> np.ndarray:
    y16 = np.concatenate([r["y"] for r in results], axis=0)  # [16,NG,2,128,G*256]
    # Device layout -> [n, s*C + c, i, j]:
    #   y16[n, g, e, (c q), (u r j)] ; s = 2e+u, c_full = g*G + c, i = q*G + r
    y16 = y16.reshape(16, NG, 2, G, Q, 2, G, W2)
    #                  n   g  e  c  q  u  r  j  -> n (e u) (g c) (q r) j
    y16 = y16.transpose(0, 2, 5, 1, 3, 4, 6, 7)
    return np.ascontiguousarray(y16).astype(np.float32).reshape(16, 4 * C, H // 2, W2)


def kernel(x: np.ndarray) -> np.ndarray:
    from concourse.bass_utils import run_bass_kernel_spmd

    nc = _get_nc()
    in_maps = _make_in_maps(x)
    res = run_bass_kernel_spmd(nc, in_maps, core_ids=list(range(N_CORES)))
    return _gather(res.results)


# revision 15
# speedup vs baseline: 1.1194x; 1.1194x over previous
"""2D Haar DWT (level 1) Trainium2 Bass kernel — fp16 I/O.

Input  x: [16, 64, 256, 256] f32
Output y: [16, 256, 128, 128] f32, y[n, s*64+c, i, j] = Haar mix s of the
2x2 block x[n, c, 2i:2i+2, 2j:2j+2].

Sharding: pure data parallel over the batch dim — core k gets batches
[2k, 2k+2).

The transform is pure data movement (out bytes == in bytes), so the kernel
is HBM-bound. The rel-err budget (2e-2) admits fp16, halving traffic to
33.5 MB/core. The host:
  - scales x by 0.5 (exact power of two — folds the whole Haar
    normalization, so the device does pure +/- butterflies),
  - casts to fp16,
  - de-interleaves even/odd columns to [n, c, h, 2, 128] so the device
    butterflies are unit-stride (DVE 2x_1P mode needs 16-bit dtype +
    step 1 + 4B alignment),
  - un-scrambles the device's group-major output layout while upcasting.

Measured facts driving this version (v1-v7 traces):
  - DVE total is ~71 us (16.8M butterfly outputs, fp16 2x = 241 G elem/s,
    tensor_tensor only — scalar_tensor_tensor measured 1x, GpSimd offload
    and mixed tile sizes both measured negative) and is the on-chip
    critical path once DMA overlaps properly.
  - A single HWDGE ring streams at only ~300 GB/s; two rings together
    reach ~420 GB/s. So EVERY load and store is split in half across the
    sync and scalar rings, with per-ring FIFO order
    [l0, l1, l2, s0, l3, s1, s2, s3]: all of g3's load bytes are queued
    before 3/4 of the stores, so the last load lands ~56 us, ahead of
    when DVE needs it (v6 lost 8.6 us stalled on a late last load), and
    stores flow on both rings throughout so no backlog builds for the
    tail (v6 dragged 9 us of store debt past the last DVE op).
  - First/last groups load in 1 MB chunks with per-chunk stage-1 ops so
    the head starts ~3 us after DMA-queue warmup; the last group's
    stage 2 is quartered with 0.5 MB stores to shorten the tail chain.

Layout per group of G=32 channels (Q = 128/G = 4 partitions per channel):
  it[p=(c,q), (o t j)] = x[n, c0 + p//Q, 64*(p%Q) + o, t, j]
  sd[p, (v r t j)]: v=0 rows hold s = top+bottom, v=1 rows hold d
  oadd tile = [subband 0 | subband 1], osub = [subband 2 | subband 3]
  output row i = q*G + r for row-pair r in [0, G).
"""

import sys

sys.path.insert(0, "/opt/trn_rl_repo")

import numpy as np

import concourse.bacc as bacc
import concourse.mybir as mybir
from concourse.tile import TileContext

N_CORES = 8
N_PER_CORE = 2  # batches per core
C = 64  # input channels
H = 256
W = 256
W2 = W // 2
G = 32  # channels per group
NG = C // G  # groups per batch item (2)
Q = 128 // G  # partitions per channel
F16 = mybir.dt.float16
FD = G * 512  # it/sd-tile free elems (16384)
HD = FD // 2  # ring-half of a load (8192 elems = 2 MB)


def build_nc():
    nc = bacc.Bacc("TRN2", target_bir_lowering=False, debug=False)
    x = nc.dram_tensor("x", [N_PER_CORE, C, H, 2, W2], F16, kind="ExternalInput")
    y = nc.dram_tensor(
        "y", [N_PER_CORE, NG, 2, 128, G * 256], F16, kind="ExternalOutput"
    )

    groups = [(n, g) for n in range(N_PER_CORE) for g in range(NG)]

    with TileContext(nc) as tc:
        with (
            tc.tile_pool(name="inpool", bufs=2) as inpool,
            tc.tile_pool(name="sdpool", bufs=2) as sdpool,
            tc.tile_pool(name="outpool", bufs=2) as outpool,
        ):
            its = {}

            def emit_load(gi, chunks):
                """Load group gi, split across both rings; `chunks` pieces
                per ring (1 = one 2 MB half per ring)."""
                n, g = groups[gi]
                it = inpool.tile([128, FD], F16, tag="in")
                its[gi] = it
                src = x[n, g * G : (g + 1) * G].rearrange(
                    "c (q o) t j -> (c q) (o t j)", q=Q
                )
                csz = HD // chunks
                for m in range(chunks):
                    for eng, base in ((nc.sync, 0), (nc.scalar, HD)):
                        lo = base + m * csz
                        eng.dma_start(
                            out=it[:, lo : lo + csz], in_=src[:, lo : lo + csz]
                        )

            def emit_stage1(gi, rs):
                """Vertical butterfly (rows 2r/2r+1) into sd, range rs."""
                it = its[gi]
                itv = it[:].rearrange("p (r u f) -> p r u f", r=G, u=2)
                sd = its[(gi, "sd")]
                sdv = sd[:].rearrange("p (v r f) -> p v r f", v=2, r=G)
                nc.vector.tensor_add(
                    out=sdv[:, 0, rs], in0=itv[:, rs, 0], in1=itv[:, rs, 1]
                )
                nc.vector.tensor_sub(
                    out=sdv[:, 1, rs], in0=itv[:, rs, 0], in1=itv[:, rs, 1]
                )

            def emit_stage2_and_stores(gi, quarters=False):
                n, g = groups[gi]
                sd = its[(gi, "sd")]
                sdt = sd[:].rearrange("p (w t j) -> p w t j", t=2, j=W2)
                oadd = outpool.tile([128, G * 256], F16, tag="oadd")
                osub = outpool.tile([128, G * 256], F16, tag="osub")
                oav = oadd[:].rearrange("p (w j) -> p w j", j=W2)
                osv = osub[:].rearrange("p (w j) -> p w j", j=W2)
                half = G * 128  # elems per v-half of an output tile
                if not quarters:
                    nc.vector.tensor_add(out=oav, in0=sdt[:, :, 0], in1=sdt[:, :, 1])
                    nc.vector.tensor_sub(out=osv, in0=sdt[:, :, 0], in1=sdt[:, :, 1])
                    for e, t_ in enumerate((oadd, osub)):
                        nc.sync.dma_start(out=y[n, g, e, :, :half], in_=t_[:, :half])
                        nc.scalar.dma_start(out=y[n, g, e, :, half:], in_=t_[:, half:])
                else:
                    # tail: quarter ops + 0.5 MB stores on both rings
                    qf = half // 2
                    for k in range(4):
                        wq = slice(k * (G // 2), (k + 1) * (G // 2))
                        nc.vector.tensor_add(
                            out=oav[:, wq], in0=sdt[:, wq, 0], in1=sdt[:, wq, 1]
                        )
                        nc.vector.tensor_sub(
                            out=osv[:, wq], in0=sdt[:, wq, 0], in1=sdt[:, wq, 1]
                        )
                        for (e, t_), eng in zip(
                            enumerate((oadd, osub)), (nc.sync, nc.scalar)
                        ):
                            eng.dma_start(
                                out=y[n, g, e, :, k * qf : (k + 1) * qf],
                                in_=t_[:, k * qf : (k + 1) * qf],
                            )

            def alloc_sd(gi):
                its[(gi, "sd")] = sdpool.tile(
                    [128, FD], F16, tag="sd", name=f"sd{gi}"
                )

            # ---- explicit schedule: per-ring DMA order is
            # [l0, l1, l2, s0, l3, s1, s2, s3]; DVE order is g0..g3.
            emit_load(0, chunks=2)  # head: 4x 1 MB chunks (2 per ring)
            emit_load(1, chunks=1)
            emit_load(2, chunks=1)
            # g0 compute: per-chunk stage 1 (ring A carries row-pairs
            # [0,16), ring B [16,32); consume both rings' chunk 0 first)
            alloc_sd(0)
            for m in range(2):
                emit_stage1(0, slice(m * 8, m * 8 + 8))
                emit_stage1(0, slice(16 + m * 8, 16 + m * 8 + 8))
            emit_stage2_and_stores(0)
            emit_load(3, chunks=2)
            alloc_sd(1)
            emit_stage1(1, slice(0, G))
            emit_stage2_and_stores(1)
            alloc_sd(2)
            emit_stage1(2, slice(0, G))
            emit_stage2_and_stores(2)
            # g3 compute: per-chunk stage 1, quartered stage 2 + stores
            alloc_sd(3)
            for m in range(2):
                emit_stage1(3, slice(m * 8, m * 8 + 8))
                emit_stage1(3, slice(16 + m * 8, 16 + m * 8 + 8))
            emit_stage2_and_stores(3, quarters=True)

    nc.finalize()
    return nc


_NC = None


def _get_nc():
    global _NC
    if _NC is None:
        _NC = build_nc()
    return _NC


def _make_in_maps(x: np.ndarray) -> list[dict]:
    """Host prep: *0.5, cast fp16, de-interleave even/odd columns."""
    x = np.asarray(x)
    assert x.shape == (16, C, H, W), x.shape
    xr = x.reshape(16, C, H, W2, 2)
    xp = np.empty((16, C, H, 2, W2), dtype=np.float16)
    np.multiply(xr[..., 0], np.float32(0.5), out=xp[:, :, :, 0, :])
    np.multiply(xr[..., 1], np.float32(0.5), out=xp[:, :, :, 1, :])
    return [
        {"x": xp[k * N_PER_CORE : (k + 1) * N_PER_CORE]} for k in range(N_CORES)
    ]


def _gather(results: list[dict]) -> np.ndarray:
    y16 = np.concatenate([r["y"] for r in results], axis=0)  # [16,NG,2,128,G*256]
    # Device layout -> [n, s*C + c, i, j]:
    #   y16[n, g, e, (c q), (v r j)] ; s = 2e+v, c_full = g*G + c, i = q*G + r
    y16 = y16.reshape(16, NG, 2, G, Q, 2, G, W2)
    #                  n   g  e  c  q  v  r  j  -> n (e v) (g c) (q r) j
    y16 = y16.transpose(0, 2, 5, 1, 3, 4, 6, 7)
    return np.ascontiguousarray(y16).astype(np.float32).reshape(16, 4 * C, H // 2, W2)


def kernel(x: np.ndarray) -> np.ndarray:
    from concourse.bass_utils import run_bass_kernel_spmd

    nc = _get_nc()
    in_maps = _make_in_maps(x)
    res = run_bass_kernel_spmd(nc, in_maps, core_ids=list(range(N_CORES)))
    return _gather(res.results)
